# revision 37
# baseline (speedup 1.0000x reference)
"""SAGEConv (mean aggregation) + ReLU on 8 Trainium2 NeuronCores.

Problem: h = relu(mean_agg(x, edges) @ W_l.T + b_l + x @ W_r.T)
  x [8, 55296, 64] f32, 221184 random edges, W [256, 64].

Strategy (dst-sharded, all-batch):
  Core c owns destination nodes [c*6912, (c+1)*6912) for ALL 8 batches.
  x re-laid host-side as node-major rows of 512 (8 batches x 64 feats) in
  fp8-e3m4, split into lo/hi tables (int16 gather-index limit).
  Selection matrices S (edge -> dst one-hot scaled by 1/deg, fp8-e3m4)
  are fully PRECOMPUTED ON HOST and streamed from HBM: no on-chip S build.
  Per core, per superblock (768 dsts = 6 groups of 128):
    - Edges PACKED densely per (sb, half) (sorted by dst), two dma_gather
      calls per half rotating over 4 SWDGE queues (concurrent Q7 descriptor
      generation); trailing slack gathers the zero row.  fp8 rows = 512 B.
    - One HWDGE DMA loads the sb's S blocks [128e, sum(touch widths)].
    - TensorE accumulates aggT[feat128, 4fc x 128dst] per group into a full
      PSUM bank; matmul N is TRIMMED to each chunk's actual dst range
      (union over cores, extended so every bank element is written once).
      ONE start=True per bank clears the whole bank's has-written bits.
    - Scaled agg PSUM is copied (Scalar engine) into packed comb tiles
      [agg64 ; x64] per batch-parity; the x half arrives via per-batch DMA
      from a host-transposed xself (bf16).
    - Phase B: one K=128 bf16 matmul per (128 dsts, batch) against stacked
      [W_l;W_r] (parity-swapped for odd batches), relu (DVE/ACT split) into
      a per-(batch, sb) staging tile, ONE bf16 output DMA per (batch, sb).
  Output: bf16 [8, 6912, 256] per core -> host concat + upcast to f32.
"""

import os
import numpy as np

_NQUEUES = int(os.environ.get("K_NQUEUES", "4"))

N_NODES = 55296
F_IN = 64
F_HID = 256
BATCH = 8
NCORE = 8
ND = N_NODES // NCORE          # 6912 dsts per core
GSZ = 128                      # dst group size (PSUM bank: 4fc x 128 dsts)
NGL = 6                        # groups per superblock
SBD = GSZ * NGL                # 768 dsts per superblock
NSB = ND // SBD                # 9 superblocks
HALF = N_NODES // 2            # 27648
EW = BATCH * F_IN              # 512 elems per node row

_cache = {}


def _build(schedule, has_bias):
    import concourse.bacc as bacc
    import concourse.tile as tile
    import concourse.mybir as mybir

    K, touches = schedule  # K[sb][h]; touches[sb] = ((ci, g, lo, hi), ...)
    bf16 = mybir.dt.bfloat16
    fp8 = mybir.dt.float8e3
    f32 = mybir.dt.float32

    sb_cols = [K[s][0] + K[s][1] for s in range(NSB)]
    max_sb_cols = max(sb_cols)
    tot_cols = sum(sb_cols)
    sb_width = [sum(t[3] - t[2] for t in touches[s]) for s in range(NSB)]
    tot_width = sum(sb_width)
    max_sb_width = max(sb_width)

    nc = bacc.Bacc(None, target_bir_lowering=False, debug=False)
    with tile.TileContext(nc) as tc:
        with tc.tile_pool(name="dram", bufs=1, space="DRAM") as dram:
            # host pre-gathered messages, partition-major: column block c
            # holds edge chunk c's 128 rows of 512 fp8 (row e on part e%128)
            m_dram = dram.tile([128, tot_cols * EW], fp8,
                               kind="ExternalInput")
            # xself[par][b4][feat][dst]: batches of one parity stacked
            xself_ev = dram.tile([4, F_IN, ND], bf16, kind="ExternalInput")
            xself_od = dram.tile([4, F_IN, ND], bf16, kind="ExternalInput")
            s_dram = dram.tile([128, tot_width], fp8, kind="ExternalInput")
            w_ev = dram.tile([128, F_HID], bf16, kind="ExternalInput")
            w_od = dram.tile([128, F_HID], bf16, kind="ExternalInput")
            if has_bias:
                bias_rep = dram.tile([128, 2 * F_HID], f32,
                                     kind="ExternalInput")
            out = dram.tile([BATCH, ND, F_HID], bf16, kind="ExternalOutput")

            with (
                tc.tile_pool(name="const", bufs=1) as constp,
                tc.tile_pool(name="msgs", bufs=3) as msgsp,
                tc.tile_pool(name="sblk", bufs=3) as sblkp,
                tc.tile_pool(name="comb", bufs=4) as combp,
                tc.tile_pool(name="hsb", bufs=14) as hsbp,
                tc.tile_pool(name="aggps", bufs=2, space="PSUM") as aggpsp,
                tc.tile_pool(name="hps", bufs=4, space="PSUM") as hpsp,
            ):
                w_ev_t = constp.tile([128, F_HID], bf16)
                nc.sync.dma_start(out=w_ev_t[:], in_=w_ev[:])
                w_od_t = constp.tile([128, F_HID], bf16)
                nc.sync.dma_start(out=w_od_t[:], in_=w_od[:])
                if has_bias:
                    bias_t = constp.tile([128, 2 * F_HID], f32)
                    nc.sync.dma_start(out=bias_t[:], in_=bias_rep[:])

                # per-sb offsets into m_dram / s_dram
                col_offs = []
                w_offs = []
                io = wo = 0
                for s in range(NSB):
                    col_offs.append(io)
                    w_offs.append(wo)
                    io += K[s][0] + K[s][1]
                    wo += sb_width[s]

                st_m3 = {}
                st_s = {}
                st_comb = {}
                relu_flip = [0]

                def issue_loads(s, parts=(0, 1, 2, 3)):
                    ncols = K[s][0] + K[s][1]
                    cuts = [ncols * i // 4 for i in range(5)]
                    for part in parts:
                        if part == 0:
                            m_t = msgsp.tile([128, max_sb_cols * EW], fp8,
                                             tag="msgs", name=f"m_{s}")
                            m3 = m_t[:].rearrange("p (c e) -> p c e", e=EW)
                            st_m3[s] = m3
                            # S blocks for this sb, one HWDGE DMA
                            s_t = sblkp.tile([128, max_sb_width], fp8,
                                             tag="sblk", name=f"s_{s}")
                            st_s[s] = s_t
                            nc.sync.dma_start(
                                out=s_t[:, 0:sb_width[s]],
                                in_=s_dram[:, w_offs[s]:
                                           w_offs[s] + sb_width[s]])
                            comb = [combp.tile([128, 4 * SBD], bf16,
                                               tag=f"comb{par}",
                                               name=f"comb{par}_{s}")
                                    for par in range(2)]
                            st_comb[s] = comb
                        m3 = st_m3[s]
                        comb = st_comb[s]
                        c0, cn = cuts[part], cuts[part + 1] - cuts[part]
                        if cn > 0:
                            a = (col_offs[s] + c0) * EW
                            b_ = (col_offs[s] + c0 + cn) * EW
                            eng = nc.sync if part % 2 == 0 else nc.scalar
                            eng.dma_start(
                                out=m3[:, c0:c0 + cn, :],
                                in_=m_dram[:, a:b_]
                                .rearrange("p (c e) -> p c e", e=EW),
                            )
                        # x half of comb: 2 SWDGE DMAs per part (Q7 idle)
                        for b in (2 * part, 2 * part + 1):
                            par, b4 = b % 2, b // 2
                            xs = xself_od if par else xself_ev
                            p0 = 64 if par == 0 else 0
                            nc.gpsimd.dma_start(
                                out=comb[par][p0:p0 + 64,
                                              b4 * SBD:(b4 + 1) * SBD],
                                in_=xs[b4, :, s * SBD:(s + 1) * SBD],
                            )

                # phase B unit: batch b, pair P (256 dsts) of sb s
                def phaseB_unit(s, b, P, hst_b):
                    comb = st_comb[s]
                    par, fc = b % 2, b // 2
                    w_t = w_od_t if par else w_ev_t
                    if P == 0:
                        hst_b[b] = hsbp.tile([128, NGL * F_HID], bf16,
                                             tag="hsb", name=f"hst_{s}_{b}")
                    hst = hst_b[b]
                    h_ps = hpsp.tile([128, 512], f32, tag="hps",
                                     name=f"hps_{s}_{b}_{P}")
                    for j in range(2):
                        dch = P * 2 + j
                        nc.tensor.matmul(
                            out=h_ps[:, j * 256:(j + 1) * 256],
                            lhsT=comb[par][:, fc * SBD + dch * 128:
                                           fc * SBD + (dch + 1) * 128],
                            rhs=w_t[:],
                            start=True,
                            stop=True,
                        )
                    if has_bias:
                        nc.vector.tensor_add(
                            out=h_ps[:], in0=h_ps[:], in1=bias_t[:])
                    if relu_flip[0] % 3 == 0:
                        nc.scalar.activation(
                            out=hst[:, P * 512:(P + 1) * 512],
                            in_=h_ps[:],
                            func=mybir.ActivationFunctionType.Relu)
                    else:
                        nc.vector.tensor_relu(
                            out=hst[:, P * 512:(P + 1) * 512],
                            in_=h_ps[:])
                    relu_flip[0] += 1
                    if P == SBD // 256 - 1:
                        r0 = s * SBD
                        # slot g*128+p -> output row p*NGL+g: partition
                        # line p covers NGL consecutive 512B rows (3KB)
                        eng = nc.sync if b % 2 == 0 else nc.scalar
                        eng.dma_start(
                            out=out[b, r0:r0 + SBD, :]
                            .rearrange("(p k) h -> p k h", k=NGL),
                            in_=hst[:].rearrange("p (k h) -> p k h",
                                                 k=NGL),
                        )

                # one superblock: agg matmuls in touch order; as soon as a
                # group pair completes, evacuate it and run its phase B for
                # all batches.  loads for sb s+2 spread at pair boundaries.
                def issue_sb(s):
                    m3 = st_m3[s]
                    s_t = st_s[s]
                    comb = st_comb[s]
                    tl = touches[s]
                    # group pair P = g//2 shares one 2-bank PSUM tile laid
                    # out [128, (fc4, gg2*128d)]: bank fc//2 holds 2 fc
                    first_p = {}
                    last_p = {}
                    locs = []
                    loc = 0
                    for ti, (ci, g, lo, hi) in enumerate(tl):
                        P = g // 2
                        if P not in first_p:
                            first_p[P] = ti
                        last_p[P] = ti
                        locs.append(loc)
                        loc += hi - lo

                    agg = {}
                    hst_b = {}
                    npart = 0
                    for ti, (ci, g, lo, hi) in enumerate(tl):
                        loc = locs[ti]
                        n = hi - lo
                        lg = (g % 2) * GSZ + lo - g * GSZ
                        P = g // 2
                        if ti == first_p[P]:
                            agg[P] = aggpsp.tile([128, 1024], f32,
                                                 tag="agg",
                                                 name=f"agg_{s}_{P}")
                        a3 = agg[P][:].rearrange("p (f d) -> p f d", f=4)
                        for fc in range(4):
                            nc.tensor.matmul(
                                out=a3[:, fc, lg:lg + n],
                                lhsT=m3[:, ci, fc * 128:(fc + 1) * 128],
                                rhs=s_t[:, loc:loc + n],
                                start=(ti == first_p[P] and fc % 2 == 0),
                                stop=(ti == last_p[P] and fc % 2 == 1),
                                skip_group_check=True,
                            )
                        if ti != last_p[P]:
                            continue
                        # evacuate pair P (256 dsts) into comb (per parity)
                        a4 = agg[P][:].rearrange("p (f d) -> p f d", f=4)
                        c4 = [comb[par][:].rearrange("p (f d) -> p f d",
                                                     f=4)
                              for par in range(2)]
                        dsl = slice(P * 256, (P + 1) * 256)
                        nc.scalar.activation(
                            out=c4[0][0:64, :, dsl],
                            in_=a4[0:64, :, :],
                            func=mybir.ActivationFunctionType.Copy)
                        nc.scalar.activation(
                            out=c4[1][64:128, :, dsl],
                            in_=a4[64:128, :, :],
                            func=mybir.ActivationFunctionType.Copy)
                        for b in range(BATCH):
                            phaseB_unit(s, b, P, hst_b)
                            if b % 4 == 3 and s + 2 < NSB and npart < 4:
                                issue_loads(s + 2, (npart,))
                                npart += 1

                issue_loads(0)
                issue_loads(1)
                for s in range(NSB):
                    issue_sb(s)
    nc.compile()
    names = dict(
        m_dram=m_dram.name,
        xself_ev=xself_ev.name, xself_od=xself_od.name,
        s_dram=s_dram.name, w_ev=w_ev.name, w_od=w_od.name,
        out=out.name, bias_rep=(bias_rep.name if has_bias else None),
    )
    return nc, names


def _prep(x, edge_src, edge_dst, W_l, b_l, W_r):
    from ml_dtypes import bfloat16, float8_e3m4

    deg = np.bincount(edge_dst, minlength=N_NODES)
    inv8 = (1.0 / np.maximum(deg, 1.0).astype(np.float32)).astype(
        float8_e3m4)

    xn = np.ascontiguousarray(x.transpose(1, 0, 2)).reshape(N_NODES, EW)
    xn8 = xn.astype(float8_e3m4)

    # dst "slot" permutation within each superblock: node with local id
    # l (within sb) occupies kernel slot sigma = (l % NGL)*GSZ + l // NGL.
    # Then slot sigma = g*128 + p outputs to row p*NGL + g, so each SBUF
    # partition line holds NGL consecutive output rows (3KB-contiguous
    # output DMA descriptors).  perm[sigma] = l  (slot -> local node).
    sig = np.arange(SBD)
    perm_sb = (sig % GSZ) * NGL + sig // GSZ      # slot -> local node id
    inv_sb = np.empty(SBD, np.int64)
    inv_sb[perm_sb] = sig                          # local node -> slot

    core = edge_dst // ND
    per_core = []
    cnt = np.zeros((NCORE, NSB, 2), np.int64)
    for c in range(NCORE):
        sel = core == c
        edl = (edge_dst[sel] - c * ND).astype(np.int64)
        ed = (edl // SBD) * SBD + inv_sb[edl % SBD]   # slot-space dst
        es = edge_src[sel].astype(np.int64)
        sb = ed // SBD
        h = (es >= HALF).astype(np.int64)
        order = np.lexsort((es, ed, h, sb))
        ed, es, sb, h = ed[order], es[order], sb[order], h[order]
        key = sb * 2 + h
        bounds = np.searchsorted(key, np.arange(2 * NSB + 1))
        cnt[c] = np.diff(bounds).reshape(NSB, 2)
        per_core.append((ed, es, bounds))

    Kmat = np.ceil(cnt.max(axis=0) / 128).astype(np.int64)
    Kmat = np.maximum(Kmat, 1)
    K = tuple((int(Kmat[s, 0]), int(Kmat[s, 1])) for s in range(NSB))

    # per-core local dst per chunk [ncols, 128] (pad -> -1), and the
    # union-over-cores dst range [lo, hi) per chunk
    ncols_s = [int(Kmat[s, 0] + Kmat[s, 1]) for s in range(NSB)]
    dl_core = []           # dl_core[c][s] = [ncols, 128] int
    lo_arr = [np.full(ncols_s[s], SBD, np.int64) for s in range(NSB)]
    hi_arr = [np.full(ncols_s[s], -1, np.int64) for s in range(NSB)]
    for c in range(NCORE):
        ed, es, bounds = per_core[c]
        dls = []
        for s in range(NSB):
            ncols = ncols_s[s]
            dl = np.full((ncols, 128), -1, np.int64)
            ci = 0
            for h in range(2):
                lo_b, hi_b = bounds[2 * s + h], bounds[2 * s + h + 1]
                n = hi_b - lo_b
                kk = int(Kmat[s, h])
                loc = ed[lo_b:hi_b] - s * SBD
                for k in range(kk):
                    a, b = k * 128, min((k + 1) * 128, n)
                    if a < n:
                        dl[ci, 0:b - a] = loc[a:b]
                        lo_arr[s][ci] = min(lo_arr[s][ci], loc[a])
                        hi_arr[s][ci] = max(hi_arr[s][ci], loc[b - 1])
                    ci += 1
            dls.append(dl)
        dl_core.append(dls)

    # shared touch list per sb: (ci, g, lo, hi), trimmed + coverage-extended
    touches = []
    for s in range(NSB):
        tl = []
        for ci in range(ncols_s[s]):
            lo = int(lo_arr[s][ci])
            hi = int(hi_arr[s][ci]) + 1
            if hi <= 0:  # chunk empty on every core (can't happen, but safe)
                lo, hi = 0, 2
            lo = (lo // 2) * 2
            hi = min(SBD, ((hi + 1) // 2) * 2)
            for g in range(lo // GSZ, (hi - 1) // GSZ + 1):
                a = max(lo, g * GSZ)
                b = min(hi, (g + 1) * GSZ)
                tl.append([ci, g, a, b])
        cov = np.zeros(SBD, bool)
        for (_, _, a, b) in tl:
            cov[a:b] = True
        for g in range(NGL):
            base = g * GSZ
            seg = cov[base:base + GSZ]
            if seg.all():
                continue
            gt = [t for t in tl if t[1] == g]
            if not gt:
                tl.append([0, g, base, base + GSZ])
                continue
            idx = np.flatnonzero(~seg)
            t0 = gt[0]
            t0[2] = min(t0[2], (base + int(idx.min())) // 2 * 2)
            t0[3] = max(t0[3], min(base + GSZ,
                                   ((base + int(idx.max()) + 2) // 2) * 2))
        tl.sort(key=lambda t: (t[0], t[1]))
        touches.append(tuple((int(a), int(b), int(cc), int(d))
                             for (a, b, cc, d) in tl))
    touches = tuple(touches)
    schedule = (K, touches)

    sb_width = [sum(t[3] - t[2] for t in touches[s]) for s in range(NSB)]
    tot_width = sum(sb_width)

    WlT = W_l.T.astype(np.float32)
    WrT = W_r.T.astype(np.float32)
    w_ev = np.vstack([WlT, WrT]).astype(bfloat16)
    w_od = np.vstack([WrT, WlT]).astype(bfloat16)
    has_bias = bool(np.any(b_l != 0))
    bias_rep = (np.broadcast_to(
        np.tile(b_l.astype(np.float32), 2)[None, :],
        (128, 2 * F_HID)).copy() if has_bias else None)

    tot_cols = int(Kmat.sum())

    in_maps = []
    for c in range(NCORE):
        ed, es, bounds = per_core[c]
        # host pre-gathered messages, edge-chunk order, pad rows zero
        msgs = np.zeros((tot_cols * 128, EW), float8_e3m4)
        row = 0
        for s in range(NSB):
            for h in range(2):
                kk = int(Kmat[s, h])
                if kk == 0:
                    continue
                lo_b, hi_b = bounds[2 * s + h], bounds[2 * s + h + 1]
                n = hi_b - lo_b
                msgs[row:row + n] = xn8[es[lo_b:hi_b]]
                row += kk * 128
        # partition-major: [128, tot_cols*EW], edge e of chunk c on
        # partition e, columns [c*EW, (c+1)*EW)
        m_arr = np.ascontiguousarray(
            msgs.reshape(tot_cols, 128, EW).transpose(1, 0, 2)
            .reshape(128, tot_cols * EW))

        # host-built S: one [128, hi-lo] fp8 block per touch, concatenated
        node_perm = (np.arange(ND) // SBD) * SBD + perm_sb[
            np.arange(ND) % SBD]                  # slot -> local node
        inv_loc = inv8[c * ND:(c + 1) * ND].astype(np.float32)[node_perm]
        s_f32 = np.zeros((128, tot_width), np.float32)
        off = 0
        for s in range(NSB):
            for (ci, g, lo, hi) in touches[s]:
                dvec = dl_core[c][s][ci]
                m = (dvec >= lo) & (dvec < hi)
                p = np.flatnonzero(m)
                if p.size:
                    s_f32[p, off + dvec[p] - lo] = inv_loc[s * SBD + dvec[p]]
                off += hi - lo
        s_arr = np.ascontiguousarray(s_f32.astype(float8_e3m4))

        xcT = np.ascontiguousarray(
            xn[c * ND:(c + 1) * ND][node_perm].astype(bfloat16).T).reshape(
                BATCH, F_IN, ND)
        xself_ev = np.ascontiguousarray(xcT[0::2])       # [4, 64, ND]
        xself_od = np.ascontiguousarray(xcT[1::2])

        in_maps.append(dict(
            m_dram=m_arr,
            xself_ev=xself_ev, xself_od=xself_od,
            s_dram=s_arr,
            w_ev=w_ev, w_od=w_od, bias_rep=bias_rep,
        ))
    return schedule, has_bias, in_maps


def kernel(x, edge_src, edge_dst, W_l, b_l, W_r):
    from concourse.bass_utils import run_bass_kernel_spmd

    x = np.asarray(x, dtype=np.float32)
    edge_src = np.asarray(edge_src, dtype=np.int32)
    edge_dst = np.asarray(edge_dst, dtype=np.int32)
    W_l = np.asarray(W_l, dtype=np.float32)
    b_l = np.asarray(b_l, dtype=np.float32)
    W_r = np.asarray(W_r, dtype=np.float32)

    schedule, has_bias, in_maps = _prep(x, edge_src, edge_dst, W_l, b_l, W_r)
    key = (schedule, has_bias)
    if key not in _cache:
        _cache[key] = _build(schedule, has_bias)
    nc, names = _cache[key]

    run_maps = []
    for m in in_maps:
        rm = {names[k]: v for k, v in m.items()
              if names.get(k) is not None and v is not None}
        run_maps.append(rm)
    res = run_bass_kernel_spmd(nc, run_maps, list(range(NCORE)))
    outs = [np.asarray(res.results[c][names["out"]]) for c in range(NCORE)]
    return np.concatenate(outs, axis=1).astype(np.float32)


# revision 41
# speedup vs baseline: 1.0403x; 1.0403x over previous
"""SAGEConv (mean aggregation) + ReLU on 8 Trainium2 NeuronCores.

Problem: h = relu(mean_agg(x, edges) @ W_l.T + b_l + x @ W_r.T)
  x [8, 55296, 64] f32, 221184 random edges, W [256, 64].

Strategy (dst-sharded, all-batch):
  Core c owns destination nodes [c*6912, (c+1)*6912) for ALL 8 batches.
  x re-laid host-side as node-major rows of 512 (8 batches x 64 feats) in
  fp8-e3m4, split into lo/hi tables (int16 gather-index limit).
  Selection matrices S (edge -> dst one-hot scaled by 1/deg, fp8-e3m4)
  are fully PRECOMPUTED ON HOST and streamed from HBM: no on-chip S build.
  Per core, per superblock (768 dsts = 6 groups of 128):
    - Edges PACKED densely per (sb, half) (sorted by dst), two dma_gather
      calls per half rotating over 4 SWDGE queues (concurrent Q7 descriptor
      generation); trailing slack gathers the zero row.  fp8 rows = 512 B.
    - One HWDGE DMA loads the sb's S blocks [128e, sum(touch widths)].
    - TensorE accumulates aggT[feat128, 4fc x 128dst] per group into a full
      PSUM bank; matmul N is TRIMMED to each chunk's actual dst range
      (union over cores, extended so every bank element is written once).
      ONE start=True per bank clears the whole bank's has-written bits.
    - Scaled agg PSUM is copied (Scalar engine) into packed comb tiles
      [agg64 ; x64] per batch-parity; the x half arrives via per-batch DMA
      from a host-transposed xself (bf16).
    - Phase B: one K=128 bf16 matmul per (128 dsts, batch) against stacked
      [W_l;W_r] (parity-swapped for odd batches), relu (DVE/ACT split) into
      a per-(batch, sb) staging tile, ONE bf16 output DMA per (batch, sb).
  Output: bf16 [8, 6912, 256] per core -> host concat + upcast to f32.
"""

import os
import numpy as np

_NQUEUES = int(os.environ.get("K_NQUEUES", "4"))

N_NODES = 55296
F_IN = 64
F_HID = 256
BATCH = 8
NCORE = 8
ND = N_NODES // NCORE          # 6912 dsts per core
GSZ = 128                      # dst group size (PSUM bank: 4fc x 128 dsts)
NGL = 6                        # groups per superblock
SBD = GSZ * NGL                # 768 dsts per superblock
NSB = ND // SBD                # 9 superblocks
HALF = N_NODES // 2            # 27648
EW = BATCH * F_IN              # 512 elems per node row

_cache = {}


def _build(schedule, has_bias):
    import concourse.bacc as bacc
    import concourse.tile as tile
    import concourse.mybir as mybir

    K, touches = schedule  # K[sb][h]; touches[sb] = ((ci, g, lo, hi), ...)
    bf16 = mybir.dt.bfloat16
    fp8 = mybir.dt.float8e3
    f32 = mybir.dt.float32

    sb_cols = [K[s][0] + K[s][1] for s in range(NSB)]
    max_sb_cols = max(sb_cols)
    tot_cols = sum(sb_cols)
    sb_width = [sum(t[3] - t[2] for t in touches[s]) for s in range(NSB)]
    tot_width = sum(sb_width)
    max_sb_width = max(sb_width)

    nc = bacc.Bacc(None, target_bir_lowering=False, debug=False)
    with tile.TileContext(nc) as tc:
        with tc.tile_pool(name="dram", bufs=1, space="DRAM") as dram:
            # host pre-gathered messages, partition-major: column block c
            # holds edge chunk c's 128 rows of 512 fp8 (row e on part e%128)
            m_dram = dram.tile([128, tot_cols * EW], fp8,
                               kind="ExternalInput")
            # xself[par][b4][feat][dst]: batches of one parity stacked
            xself_ev = dram.tile([4, F_IN, ND], bf16, kind="ExternalInput")
            xself_od = dram.tile([4, F_IN, ND], bf16, kind="ExternalInput")
            s_dram = dram.tile([128, tot_width], fp8, kind="ExternalInput")
            w_ev = dram.tile([128, F_HID], bf16, kind="ExternalInput")
            w_od = dram.tile([128, F_HID], bf16, kind="ExternalInput")
            if has_bias:
                bias_rep = dram.tile([128, 2 * F_HID], f32,
                                     kind="ExternalInput")
            out = dram.tile([BATCH, ND, F_HID], bf16, kind="ExternalOutput")

            with (
                tc.tile_pool(name="const", bufs=1) as constp,
                tc.tile_pool(name="msgs", bufs=3) as msgsp,
                tc.tile_pool(name="sblk", bufs=3) as sblkp,
                tc.tile_pool(name="comb", bufs=4) as combp,
                tc.tile_pool(name="hsb", bufs=16) as hsbp,
                tc.tile_pool(name="aggps", bufs=2, space="PSUM") as aggpsp,
                tc.tile_pool(name="hps", bufs=4, space="PSUM") as hpsp,
            ):
                w_ev_t = constp.tile([128, F_HID], bf16)
                nc.sync.dma_start(out=w_ev_t[:], in_=w_ev[:])
                w_od_t = constp.tile([128, F_HID], bf16)
                nc.sync.dma_start(out=w_od_t[:], in_=w_od[:])
                if has_bias:
                    bias_t = constp.tile([128, 2 * F_HID], f32)
                    nc.sync.dma_start(out=bias_t[:], in_=bias_rep[:])

                # per-sb offsets into m_dram / s_dram
                col_offs = []
                w_offs = []
                io = wo = 0
                for s in range(NSB):
                    col_offs.append(io)
                    w_offs.append(wo)
                    io += K[s][0] + K[s][1]
                    wo += sb_width[s]

                st_m3 = {}
                st_s = {}
                st_comb = {}
                relu_flip = [0]

                def issue_loads(s, parts=(0, 1, 2, 3)):
                    ncols = K[s][0] + K[s][1]
                    cuts = [ncols * i // 4 for i in range(5)]
                    for part in parts:
                        if part == 0:
                            m_t = msgsp.tile([128, max_sb_cols * EW], fp8,
                                             tag="msgs", name=f"m_{s}")
                            m3 = m_t[:].rearrange("p (c e) -> p c e", e=EW)
                            st_m3[s] = m3
                            # S blocks for this sb, one HWDGE DMA
                            s_t = sblkp.tile([128, max_sb_width], fp8,
                                             tag="sblk", name=f"s_{s}")
                            st_s[s] = s_t
                            nc.sync.dma_start(
                                out=s_t[:, 0:sb_width[s]],
                                in_=s_dram[:, w_offs[s]:
                                           w_offs[s] + sb_width[s]])
                            comb = [combp.tile([128, 4 * SBD], bf16,
                                               tag=f"comb{par}",
                                               name=f"comb{par}_{s}")
                                    for par in range(2)]
                            st_comb[s] = comb
                        m3 = st_m3[s]
                        comb = st_comb[s]
                        c0, cn = cuts[part], cuts[part + 1] - cuts[part]
                        if cn > 0:
                            a = (col_offs[s] + c0) * EW
                            b_ = (col_offs[s] + c0 + cn) * EW
                            eng = nc.sync if part % 2 == 0 else nc.scalar
                            eng.dma_start(
                                out=m3[:, c0:c0 + cn, :],
                                in_=m_dram[:, a:b_]
                                .rearrange("p (c e) -> p c e", e=EW),
                            )
                        # x half of comb: 2 SWDGE DMAs per part (Q7 idle)
                        for b in (2 * part, 2 * part + 1):
                            par, b4 = b % 2, b // 2
                            xs = xself_od if par else xself_ev
                            p0 = 64 if par == 0 else 0
                            nc.gpsimd.dma_start(
                                out=comb[par][p0:p0 + 64,
                                              b4 * SBD:(b4 + 1) * SBD],
                                in_=xs[b4, :, s * SBD:(s + 1) * SBD],
                            )

                # phase B unit: batch b, pair P (256 dsts) of sb s
                def phaseB_unit(s, b, P, hst_b):
                    comb = st_comb[s]
                    par, fc = b % 2, b // 2
                    w_t = w_od_t if par else w_ev_t
                    if P == 0:
                        hst_b[b] = hsbp.tile([128, NGL * F_HID], bf16,
                                             tag="hsb", name=f"hst_{s}_{b}")
                    hst = hst_b[b]
                    h_ps = hpsp.tile([128, 512], f32, tag="hps",
                                     name=f"hps_{s}_{b}_{P}")
                    for j in range(2):
                        dch = P * 2 + j
                        nc.tensor.matmul(
                            out=h_ps[:, j * 256:(j + 1) * 256],
                            lhsT=comb[par][:, fc * SBD + dch * 128:
                                           fc * SBD + (dch + 1) * 128],
                            rhs=w_t[:],
                            start=True,
                            stop=True,
                        )
                    if has_bias:
                        nc.vector.tensor_add(
                            out=h_ps[:], in0=h_ps[:], in1=bias_t[:])
                    if relu_flip[0] % 3 == 0:
                        nc.scalar.activation(
                            out=hst[:, P * 512:(P + 1) * 512],
                            in_=h_ps[:],
                            func=mybir.ActivationFunctionType.Relu)
                    else:
                        nc.vector.tensor_relu(
                            out=hst[:, P * 512:(P + 1) * 512],
                            in_=h_ps[:])
                    relu_flip[0] += 1
                    if P == SBD // 256 - 1:
                        r0 = s * SBD
                        # slot g*128+p -> output row p*NGL+g: partition
                        # line p covers NGL consecutive 512B rows (3KB)
                        eng = nc.sync if b % 2 == 0 else nc.scalar
                        eng.dma_start(
                            out=out[b, r0:r0 + SBD, :]
                            .rearrange("(p k) h -> p k h", k=NGL),
                            in_=hst[:].rearrange("p (k h) -> p k h",
                                                 k=NGL),
                        )

                # pending completed pairs; phase B runs one pair behind
                # evacuation so the PE never waits on the scalar evac
                pending = []
                st_hst = {}
                part_state = {}

                def flush_pb():
                    (ps, pP) = pending.pop(0)
                    for b in range(BATCH):
                        phaseB_unit(ps, b, pP, st_hst[ps])
                        if b % 4 == 3:
                            s2, npart = part_state.get(ps, (ps + 2, 4))
                            if s2 < NSB and npart < 4:
                                issue_loads(s2, (npart,))
                                part_state[ps] = (s2, npart + 1)

                # one superblock: agg matmuls in touch order; as soon as a
                # group pair completes, evacuate it and run phase B of the
                # previously completed pair.  loads spread at pb slots.
                def issue_sb(s):
                    m3 = st_m3[s]
                    s_t = st_s[s]
                    comb = st_comb[s]
                    tl = touches[s]
                    # group pair P = g//2 shares one 2-bank PSUM tile laid
                    # out [128, (fc4, gg2*128d)]: bank fc//2 holds 2 fc
                    first_p = {}
                    last_p = {}
                    locs = []
                    loc = 0
                    for ti, (ci, g, lo, hi) in enumerate(tl):
                        P = g // 2
                        if P not in first_p:
                            first_p[P] = ti
                        last_p[P] = ti
                        locs.append(loc)
                        loc += hi - lo

                    agg = {}
                    st_hst[s] = {}
                    part_state[s] = (s + 2, 0)
                    for ti, (ci, g, lo, hi) in enumerate(tl):
                        loc = locs[ti]
                        n = hi - lo
                        lg = (g % 2) * GSZ + lo - g * GSZ
                        P = g // 2
                        if ti == first_p[P]:
                            agg[P] = aggpsp.tile([128, 1024], f32,
                                                 tag="agg",
                                                 name=f"agg_{s}_{P}")
                        a3 = agg[P][:].rearrange("p (f d) -> p f d", f=4)
                        for fc in range(4):
                            nc.tensor.matmul(
                                out=a3[:, fc, lg:lg + n],
                                lhsT=m3[:, ci, fc * 128:(fc + 1) * 128],
                                rhs=s_t[:, loc:loc + n],
                                start=(ti == first_p[P] and fc % 2 == 0),
                                stop=(ti == last_p[P] and fc % 2 == 1),
                                skip_group_check=True,
                            )
                        if ti != last_p[P]:
                            continue
                        # evacuate pair P (256 dsts) into comb (per parity)
                        a4 = agg[P][:].rearrange("p (f d) -> p f d", f=4)
                        c4 = [comb[par][:].rearrange("p (f d) -> p f d",
                                                     f=4)
                              for par in range(2)]
                        dsl = slice(P * 256, (P + 1) * 256)
                        nc.scalar.activation(
                            out=c4[0][0:64, :, dsl],
                            in_=a4[0:64, :, :],
                            func=mybir.ActivationFunctionType.Copy)
                        nc.scalar.activation(
                            out=c4[1][64:128, :, dsl],
                            in_=a4[64:128, :, :],
                            func=mybir.ActivationFunctionType.Copy)
                        pending.append((s, P))
                        if len(pending) > 1:
                            flush_pb()

                issue_loads(0)
                issue_loads(1)
                for s in range(NSB):
                    issue_sb(s)
                while pending:
                    flush_pb()
    nc.compile()
    names = dict(
        m_dram=m_dram.name,
        xself_ev=xself_ev.name, xself_od=xself_od.name,
        s_dram=s_dram.name, w_ev=w_ev.name, w_od=w_od.name,
        out=out.name, bias_rep=(bias_rep.name if has_bias else None),
    )
    return nc, names


def _prep(x, edge_src, edge_dst, W_l, b_l, W_r):
    from ml_dtypes import bfloat16, float8_e3m4

    deg = np.bincount(edge_dst, minlength=N_NODES)
    inv8 = (1.0 / np.maximum(deg, 1.0).astype(np.float32)).astype(
        float8_e3m4)

    xn = np.ascontiguousarray(x.transpose(1, 0, 2)).reshape(N_NODES, EW)
    xn8 = xn.astype(float8_e3m4)

    # dst "slot" permutation within each superblock: node with local id
    # l (within sb) occupies kernel slot sigma = (l % NGL)*GSZ + l // NGL.
    # Then slot sigma = g*128 + p outputs to row p*NGL + g, so each SBUF
    # partition line holds NGL consecutive output rows (3KB-contiguous
    # output DMA descriptors).  perm[sigma] = l  (slot -> local node).
    sig = np.arange(SBD)
    perm_sb = (sig % GSZ) * NGL + sig // GSZ      # slot -> local node id
    inv_sb = np.empty(SBD, np.int64)
    inv_sb[perm_sb] = sig                          # local node -> slot

    core = edge_dst // ND
    per_core = []
    cnt = np.zeros((NCORE, NSB, 2), np.int64)
    for c in range(NCORE):
        sel = core == c
        edl = (edge_dst[sel] - c * ND).astype(np.int64)
        ed = (edl // SBD) * SBD + inv_sb[edl % SBD]   # slot-space dst
        es = edge_src[sel].astype(np.int64)
        sb = ed // SBD
        h = (es >= HALF).astype(np.int64)
        order = np.lexsort((es, ed, h, sb))
        ed, es, sb, h = ed[order], es[order], sb[order], h[order]
        key = sb * 2 + h
        bounds = np.searchsorted(key, np.arange(2 * NSB + 1))
        cnt[c] = np.diff(bounds).reshape(NSB, 2)
        per_core.append((ed, es, bounds))

    Kmat = np.ceil(cnt.max(axis=0) / 128).astype(np.int64)
    Kmat = np.maximum(Kmat, 1)
    K = tuple((int(Kmat[s, 0]), int(Kmat[s, 1])) for s in range(NSB))

    # per-core local dst per chunk [ncols, 128] (pad -> -1), and the
    # union-over-cores dst range [lo, hi) per chunk
    ncols_s = [int(Kmat[s, 0] + Kmat[s, 1]) for s in range(NSB)]
    dl_core = []           # dl_core[c][s] = [ncols, 128] int
    lo_arr = [np.full(ncols_s[s], SBD, np.int64) for s in range(NSB)]
    hi_arr = [np.full(ncols_s[s], -1, np.int64) for s in range(NSB)]
    for c in range(NCORE):
        ed, es, bounds = per_core[c]
        dls = []
        for s in range(NSB):
            ncols = ncols_s[s]
            dl = np.full((ncols, 128), -1, np.int64)
            ci = 0
            for h in range(2):
                lo_b, hi_b = bounds[2 * s + h], bounds[2 * s + h + 1]
                n = hi_b - lo_b
                kk = int(Kmat[s, h])
                loc = ed[lo_b:hi_b] - s * SBD
                for k in range(kk):
                    a, b = k * 128, min((k + 1) * 128, n)
                    if a < n:
                        dl[ci, 0:b - a] = loc[a:b]
                        lo_arr[s][ci] = min(lo_arr[s][ci], loc[a])
                        hi_arr[s][ci] = max(hi_arr[s][ci], loc[b - 1])
                    ci += 1
            dls.append(dl)
        dl_core.append(dls)

    # shared touch list per sb: (ci, g, lo, hi), trimmed + coverage-extended
    touches = []
    for s in range(NSB):
        tl = []
        for ci in range(ncols_s[s]):
            lo = int(lo_arr[s][ci])
            hi = int(hi_arr[s][ci]) + 1
            if hi <= 0:  # chunk empty on every core (can't happen, but safe)
                lo, hi = 0, 2
            lo = (lo // 2) * 2
            hi = min(SBD, ((hi + 1) // 2) * 2)
            for g in range(lo // GSZ, (hi - 1) // GSZ + 1):
                a = max(lo, g * GSZ)
                b = min(hi, (g + 1) * GSZ)
                tl.append([ci, g, a, b])
        cov = np.zeros(SBD, bool)
        for (_, _, a, b) in tl:
            cov[a:b] = True
        for g in range(NGL):
            base = g * GSZ
            seg = cov[base:base + GSZ]
            if seg.all():
                continue
            gt = [t for t in tl if t[1] == g]
            if not gt:
                tl.append([0, g, base, base + GSZ])
                continue
            idx = np.flatnonzero(~seg)
            t0 = gt[0]
            t0[2] = min(t0[2], (base + int(idx.min())) // 2 * 2)
            t0[3] = max(t0[3], min(base + GSZ,
                                   ((base + int(idx.max()) + 2) // 2) * 2))
        tl.sort(key=lambda t: (t[0], t[1]))
        touches.append(tuple((int(a), int(b), int(cc), int(d))
                             for (a, b, cc, d) in tl))
    touches = tuple(touches)
    schedule = (K, touches)

    sb_width = [sum(t[3] - t[2] for t in touches[s]) for s in range(NSB)]
    tot_width = sum(sb_width)

    WlT = W_l.T.astype(np.float32)
    WrT = W_r.T.astype(np.float32)
    w_ev = np.vstack([WlT, WrT]).astype(bfloat16)
    w_od = np.vstack([WrT, WlT]).astype(bfloat16)
    has_bias = bool(np.any(b_l != 0))
    bias_rep = (np.broadcast_to(
        np.tile(b_l.astype(np.float32), 2)[None, :],
        (128, 2 * F_HID)).copy() if has_bias else None)

    tot_cols = int(Kmat.sum())

    in_maps = []
    for c in range(NCORE):
        ed, es, bounds = per_core[c]
        # host pre-gathered messages, edge-chunk order, pad rows zero
        msgs = np.zeros((tot_cols * 128, EW), float8_e3m4)
        row = 0
        for s in range(NSB):
            for h in range(2):
                kk = int(Kmat[s, h])
                if kk == 0:
                    continue
                lo_b, hi_b = bounds[2 * s + h], bounds[2 * s + h + 1]
                n = hi_b - lo_b
                msgs[row:row + n] = xn8[es[lo_b:hi_b]]
                row += kk * 128
        # partition-major: [128, tot_cols*EW], edge e of chunk c on
        # partition e, columns [c*EW, (c+1)*EW)
        m_arr = np.ascontiguousarray(
            msgs.reshape(tot_cols, 128, EW).transpose(1, 0, 2)
            .reshape(128, tot_cols * EW))

        # host-built S: one [128, hi-lo] fp8 block per touch, concatenated
        node_perm = (np.arange(ND) // SBD) * SBD + perm_sb[
            np.arange(ND) % SBD]                  # slot -> local node
        inv_loc = inv8[c * ND:(c + 1) * ND].astype(np.float32)[node_perm]
        s_f32 = np.zeros((128, tot_width), np.float32)
        off = 0
        for s in range(NSB):
            for (ci, g, lo, hi) in touches[s]:
                dvec = dl_core[c][s][ci]
                m = (dvec >= lo) & (dvec < hi)
                p = np.flatnonzero(m)
                if p.size:
                    s_f32[p, off + dvec[p] - lo] = inv_loc[s * SBD + dvec[p]]
                off += hi - lo
        s_arr = np.ascontiguousarray(s_f32.astype(float8_e3m4))

        xcT = np.ascontiguousarray(
            xn[c * ND:(c + 1) * ND][node_perm].astype(bfloat16).T).reshape(
                BATCH, F_IN, ND)
        xself_ev = np.ascontiguousarray(xcT[0::2])       # [4, 64, ND]
        xself_od = np.ascontiguousarray(xcT[1::2])

        in_maps.append(dict(
            m_dram=m_arr,
            xself_ev=xself_ev, xself_od=xself_od,
            s_dram=s_arr,
            w_ev=w_ev, w_od=w_od, bias_rep=bias_rep,
        ))
    return schedule, has_bias, in_maps


def kernel(x, edge_src, edge_dst, W_l, b_l, W_r):
    from concourse.bass_utils import run_bass_kernel_spmd

    x = np.asarray(x, dtype=np.float32)
    edge_src = np.asarray(edge_src, dtype=np.int32)
    edge_dst = np.asarray(edge_dst, dtype=np.int32)
    W_l = np.asarray(W_l, dtype=np.float32)
    b_l = np.asarray(b_l, dtype=np.float32)
    W_r = np.asarray(W_r, dtype=np.float32)

    schedule, has_bias, in_maps = _prep(x, edge_src, edge_dst, W_l, b_l, W_r)
    key = (schedule, has_bias)
    if key not in _cache:
        _cache[key] = _build(schedule, has_bias)
    nc, names = _cache[key]

    run_maps = []
    for m in in_maps:
        rm = {names[k]: v for k, v in m.items()
              if names.get(k) is not None and v is not None}
        run_maps.append(rm)
    res = run_bass_kernel_spmd(nc, run_maps, list(range(NCORE)))
    outs = [np.asarray(res.results[c][names["out"]]) for c in range(NCORE)]
    return np.concatenate(outs, axis=1).astype(np.float32)
